# revision 28
# baseline (speedup 1.0000x reference)
"""Batched same-batch KNN (top-3) + fused MLP for Trainium2, 8 NeuronCores.

Strategy
--------
Host side (numpy, exact):
  * Stable-group rows of a and b by batch id. Batch g -> core g (B == 8 ==
    n_cores). Within a batch the original relative order is preserved, so
    the device's first-occurrence tie handling matches jax.lax.top_k.
  * Augment voxel coords so the [Na_g, Nb_g] block of NEGATED squared
    distances is one K=5 matmul:
        d'[i,j] = ua_i . vb_j,  ua = [-|xa|^2, 1, 2*xa],  vb = [1, -|xb|^2, xb]
    All quantities are small integers -> fp32-exact, identical to the
    reference's d (up to sign/scale).
Device side (per core, SPMD):
  * R = relu(feats_bg @ W1 + b1) precomputed once for the whole batch
    (valid because dw >= 0 and relu is positively homogeneous:
     relu(dw*t + b1)*dw == dw^2 * relu(t + b1/dw)... exactly:
     h = relu((f*dw) @ W1 + b1) * dw = dw^2 * relu(f@W1 + b1/dw) only if b1==0;
     in general we use h = relu(dw*(f@W1) + b1)*dw and the harness data has
     b1 == 0 which makes h = dw^2 * relu(f@W1). A nonzero b1 is handled by a
     numpy fallback, see kernel()).
  * Per 128-row a-tile: distance matmul -> PSUM, DVE max (top-8 desc) +
    max_index (ties resolved to successive first occurrences == jax),
    dw = relu(0.5 + d'/16384), indirect-DMA gather of 3 R rows, weighted
    sum with dw^2, PE transpose, @W2 (+3*b2), output fused.T slab.
Outputs are scattered back to original row order on host; the feats_a
passthrough half of the concat is host-side assembly.
"""

import os
import numpy as np

import concourse.bass as bass
import concourse.mybir as mybir
import concourse.tile as tile
from concourse import bacc
from concourse.bass import IndirectOffsetOnAxis
from concourse.bass_utils import run_bass_kernel_spmd
from concourse.masks import make_identity

P = 128
NPAD = 1664  # 13 * 128; covers per-batch row counts for Na=Nb=12288, B=8
NT = NPAD // P
DF = 256
TOPK = 3
FULL_SCALE = 128
RCLIP = 0.5
INV_SCALE2 = 1.0 / (FULL_SCALE * FULL_SCALE)
BIG = 1.0e9
N_CORES = 8
NQ = 3  # SWDGE queues used for indirect gathers

_PROGRAM_CACHE = {}


ROWPACK = bool(int(os.environ.get("KERNEL_ROWPACK", "1")))
BATCHGATHER = bool(int(os.environ.get("KERNEL_BATCHGATHER", "1")))


def _build_program():
    """Build the SPMD Bass program (identical on all 8 cores)."""
    nc = bacc.Bacc("TRN2", target_bir_lowering=False, debug=False)
    f32 = mybir.dt.float32

    uaT = nc.dram_tensor("uaT", [5, NPAD], f32, kind="ExternalInput").ap()
    vbT = nc.dram_tensor("vbT", [5, NPAD], f32, kind="ExternalInput").ap()
    fbT = nc.dram_tensor("fbT", [DF, NPAD], f32, kind="ExternalInput").ap()
    w1b1 = nc.dram_tensor("w1b1", [DF + 1, DF], f32, kind="ExternalInput").ap()
    w2 = nc.dram_tensor("w2", [DF, DF], f32, kind="ExternalInput").ap()
    b2c3 = nc.dram_tensor("b2c3", [P, 2], f32, kind="ExternalInput").ap()
    fusedT = nc.dram_tensor("fusedT", [DF, NPAD], f32, kind="ExternalOutput").ap()

    with tile.TileContext(nc) as tc:
        with (
            tc.tile_pool(name="const", bufs=1) as cpool,
            tc.tile_pool(name="dram", bufs=1, space="DRAM") as dpool_dram,
        ):
            # coord operands replicated at partition offsets 0/32/64/96 so the
            # K=5 distance matmuls can run in 4 concurrent PE row-groups
            uaT4 = cpool.tile([101, NPAD], f32)
            vbT4 = cpool.tile([101, NPAD], f32)
            for gofs in (0, 32, 64, 96):
                nc.sync.dma_start(uaT4[gofs : gofs + 5, :], uaT[:])
                nc.sync.dma_start(vbT4[gofs : gofs + 5, :], vbT[:])
            fbT0 = cpool.tile([P, NPAD], f32)
            nc.sync.dma_start(fbT0[:], fbT[0:P, :])
            fbT1 = cpool.tile([P, NPAD], f32)
            nc.sync.dma_start(fbT1[:], fbT[P : 2 * P, :])
            w1k0 = cpool.tile([P, DF], f32)
            nc.sync.dma_start(w1k0[:], w1b1[0:P, :])
            w1k1 = cpool.tile([P, DF], f32)
            nc.sync.dma_start(w1k1[:], w1b1[P : 2 * P, :])

            w2k0 = cpool.tile([P, DF], f32)
            nc.sync.dma_start(w2k0[:], w2[0:P, :])
            w2k1 = cpool.tile([P, DF], f32)
            nc.sync.dma_start(w2k1[:], w2[P : 2 * P, :])
            b2s = cpool.tile([P, 2], f32)
            nc.sync.dma_start(b2s[:], b2c3[:])
            ident = cpool.tile([P, P], f32)
            make_identity(nc, ident[:])
            zcol = cpool.tile([P, 1], f32)
            nc.vector.memset(zcol[:], 0.0)
            halfcol = cpool.tile([P, 1], f32)
            nc.vector.memset(halfcol[:], RCLIP)

            rtab = dpool_dram.tile([NPAD, DF], f32)

            # ---- Phase R: R = relu(feats_bg @ W1 + b1), row-major in DRAM
            with (
                tc.tile_pool(name="psR", bufs=2, space="PSUM") as psR_pool,
                tc.tile_pool(name="rsb", bufs=3) as r_pool,
            ):
                for t in range(NT):
                    sl = bass.ts(t, P)
                    psR = psR_pool.tile([P, DF], f32)
                    # b1 is asserted zero host-side (numpy fallback otherwise),
                    # so no bias term here
                    nc.tensor.matmul(
                        psR[:], lhsT=fbT0[:, sl], rhs=w1k0[:], start=True, stop=False
                    )
                    nc.tensor.matmul(
                        psR[:], lhsT=fbT1[:, sl], rhs=w1k1[:], start=False, stop=True
                    )
                    rt = r_pool.tile([P, DF], f32)
                    nc.scalar.activation(
                        rt[:], psR[:], mybir.ActivationFunctionType.Relu, bias=zcol[:]
                    )
                    nc.sync.dma_start(rtab[sl, :], rt[:])

            # ---- Phase D: distances, top-3, gather, W2 (1-stage sw pipeline:
            # tile t's top-k overlaps tile t-1's gather-consume + MLP so the
            # DVE stream never stalls on gather completion)
            with (
                tc.tile_pool(name="dps", bufs=1, space="PSUM") as d_pool,
                tc.tile_pool(name="tps", bufs=2, space="PSUM") as t_pool,
                tc.tile_pool(name="fps", bufs=2, space="PSUM") as f_pool,
                tc.tile_pool(name="dsb", bufs=2) as dsb_pool,
                tc.tile_pool(name="small", bufs=4) as s_pool,
                tc.tile_pool(name="gat", bufs=3) as g_pool,
                tc.tile_pool(name="accp", bufs=2) as a_pool,
                tc.tile_pool(name="outp", bufs=3) as o_pool,
            ):
                state = {}

                def topk_and_gather(t):
                    sl = bass.ts(t, P)
                    dps = d_pool.tile([P, NPAD], f32)
                    for c, c0 in enumerate(range(0, NPAD, 512)):
                        c1 = min(c0 + 512, NPAD)
                        gofs = 32 * c if ROWPACK else 0
                        nc.tensor.matmul(
                            dps[:, c0:c1],
                            lhsT=uaT4[gofs : gofs + 5, sl],
                            rhs=vbT4[gofs : gofs + 5, c0:c1],
                            start=True,
                            stop=True,
                            tile_position=(gofs, 0),
                        )
                    dsb = dsb_pool.tile([P, NPAD], f32)
                    nc.scalar.copy(dsb[:], dps[:])
                    vals = s_pool.tile([P, 8], f32)
                    nc.vector.max(out=vals[:], in_=dsb[:])
                    idxs = s_pool.tile([P, 8], mybir.dt.uint32)
                    nc.vector.max_index(out=idxs[:], in_max=vals[:], in_values=dsb[:])
                    dw = s_pool.tile([P, TOPK], f32)
                    nc.scalar.activation(
                        dw[:],
                        vals[:, 0:TOPK],
                        mybir.ActivationFunctionType.Relu,
                        bias=halfcol[:],
                        scale=INV_SCALE2,
                    )
                    dw2 = s_pool.tile([P, TOPK], f32)
                    nc.vector.tensor_mul(dw2[:], dw[:], dw[:])
                    gs = []
                    for k in range(TOPK):
                        g = g_pool.tile([P, DF], f32, tag=f"g{k}")
                        nc.gpsimd.indirect_dma_start(
                            out=g[:],
                            out_offset=None,
                            in_=rtab[:],
                            in_offset=IndirectOffsetOnAxis(
                                ap=idxs[:, k : k + 1], axis=0
                            ),
                        )
                        gs.append(g)
                    state[t] = (dw2, gs)

                def mlp_tail(s):
                    sl = bass.ts(s, P)
                    dw2, gs = state.pop(s)
                    acc = a_pool.tile([P, DF], f32)
                    nc.scalar.mul(acc[:], gs[0][:], dw2[:, 0:1])
                    for k in range(1, TOPK):
                        nc.vector.scalar_tensor_tensor(
                            out=acc[:],
                            in0=gs[k][:],
                            scalar=dw2[:, k : k + 1],
                            in1=acc[:],
                            op0=mybir.AluOpType.mult,
                            op1=mybir.AluOpType.add,
                        )
                    accT = []
                    for m in range(2):
                        pt = t_pool.tile([P, P], f32)
                        nc.tensor.transpose(
                            out=pt[:],
                            in_=acc[:, m * P : (m + 1) * P],
                            identity=ident[:],
                        )
                        aT = a_pool.tile([P, P], f32, tag="accT")
                        nc.scalar.copy(aT[:], pt[:])
                        accT.append(aT)
                    for m in range(2):
                        msl = bass.ts(m, P)
                        pf = f_pool.tile([P, P], f32)
                        nc.tensor.matmul(
                            pf[:],
                            lhsT=w2k0[:, msl],
                            rhs=accT[0][:],
                            start=True,
                            stop=False,
                        )
                        nc.tensor.matmul(
                            pf[:],
                            lhsT=w2k1[:, msl],
                            rhs=accT[1][:],
                            start=False,
                            stop=True,
                        )
                        oT = o_pool.tile([P, P], f32)
                        nc.scalar.activation(
                            oT[:],
                            pf[:],
                            mybir.ActivationFunctionType.Identity,
                            bias=b2s[:, m : m + 1],
                        )
                        nc.sync.dma_start(fusedT[msl, sl], oT[:])

                SKEW = 2
                for t in range(NT + SKEW):
                    if t < NT:
                        with tc.tile_wait_until(t):
                            topk_and_gather(t)
                    if t >= SKEW:
                        with tc.tile_wait_until(t + 0.5):
                            mlp_tail(t - SKEW)
    nc.compile()
    return nc


def get_program():
    if "nc" not in _PROGRAM_CACHE:
        _PROGRAM_CACHE["nc"] = _build_program()
    return _PROGRAM_CACHE["nc"]


def _host_prep(batch_a, coords_a, batch_b, coords_b, feats_b, W1, b1, W2, b2):
    """Group by batch, build per-core input arrays. Returns (in_maps, meta)."""
    pa = np.argsort(batch_a, kind="stable")
    pb = np.argsort(batch_b, kind="stable")
    ca = np.bincount(batch_a, minlength=N_CORES)
    cb = np.bincount(batch_b, minlength=N_CORES)
    oa = np.concatenate([[0], np.cumsum(ca)])
    ob = np.concatenate([[0], np.cumsum(cb)])

    w1b1 = np.concatenate([W1, b1[None, :]], axis=0).astype(np.float32)
    b2c3 = np.ascontiguousarray((3.0 * b2).astype(np.float32).reshape(2, P).T)

    in_maps = []
    meta = []
    for g in range(N_CORES):
        a_idx = pa[oa[g] : oa[g + 1]]
        b_idx = pb[ob[g] : ob[g + 1]]
        na, nb = len(a_idx), len(b_idx)
        if na > NPAD or nb > NPAD or (0 < nb < TOPK):
            return None, None  # shapes outside the compiled envelope -> fallback
        xa = (coords_a[a_idx] // 16).astype(np.float32)
        xb = (coords_b[b_idx] // 16).astype(np.float32)

        uaT = np.zeros((5, NPAD), dtype=np.float32)
        uaT[1, :] = 1.0
        if na > 0:
            uaT[0, :na] = -np.square(xa).sum(1)
            uaT[2:, :na] = (2.0 * xa).T
            # pad a-cols: copy of column 0 (harmless rows, outputs dropped)
            if na < NPAD:
                uaT[:, na:] = uaT[:, :1]

        vbT = np.zeros((5, NPAD), dtype=np.float32)
        vbT[0, :] = 1.0
        vbT[1, :] = -BIG  # pad cols: huge distance, never selected
        if nb > 0:
            vbT[1, :nb] = -np.square(xb).sum(1)
            vbT[2:, :nb] = xb.T

        fbT = np.zeros((DF, NPAD), dtype=np.float32)
        if nb > 0:
            fbT[:, :nb] = feats_b[b_idx].T

        in_maps.append(
            {
                "uaT": uaT,
                "vbT": vbT,
                "fbT": fbT,
                "w1b1": w1b1,
                "w2": np.ascontiguousarray(W2.astype(np.float32)),
                "b2c3": b2c3,
            }
        )
        meta.append((a_idx, na, nb))
    return in_maps, meta


def _reference_numpy(batch_a, coords_a, feats_a, batch_b, coords_b, feats_b,
                     W1, b1, W2, b2):
    """Exact numpy fallback (mirrors reference.py) for out-of-envelope data."""
    xa = (coords_a // 16).astype(np.float32)
    xb = (coords_b // 16).astype(np.float32)
    d = (
        np.square(xa).sum(1)[:, None]
        + np.square(xb).sum(1)[None, :]
        - 2.0 * (xa @ xb.T)
    )
    d = np.clip(d, 0.0, None) / (FULL_SCALE**2)
    same = batch_a[:, None] == batch_b[None, :]
    d = np.where(same, d, np.inf)
    idx = np.argsort(d, axis=1, kind="stable")[:, :TOPK]
    dv = np.take_along_axis(d, idx, axis=1)
    dwt = RCLIP - np.clip(dv, 0.0, RCLIP)
    b_f = feats_b[idx] * dwt[..., None]
    h = np.maximum(b_f @ W1 + b1, 0.0) * dwt[..., None]
    fused = (h @ W2 + b2).sum(axis=1)
    return np.concatenate([feats_a, fused], axis=1).astype(np.float32)


def _ensure_ntff_hook():
    """Install the axon NTFF profile hook (missing antenv.axon_hooks shim)."""
    import sys
    import types

    if "antenv.axon_hooks" in sys.modules:
        return
    try:
        from trn_agent_boot.trn_boot import _ntff_profile_via_ctypes

        hook = _ntff_profile_via_ctypes("/opt/axon/libaxon_pjrt.so")
    except Exception:
        hook = None
    mod = types.ModuleType("antenv.axon_hooks")
    _state = {"hook": hook}
    mod.get_axon_ntff_profile_hook = lambda: _state["hook"]

    def _set(h):
        _state["hook"] = h

    mod.set_axon_ntff_profile_hook = _set
    sys.modules["antenv.axon_hooks"] = mod


def kernel(batch_a, coords_a, feats_a, batch_b, coords_b, feats_b, W1, b1, W2, b2):
    batch_a = np.asarray(batch_a)
    coords_a = np.asarray(coords_a)
    feats_a = np.asarray(feats_a, dtype=np.float32)
    batch_b = np.asarray(batch_b)
    coords_b = np.asarray(coords_b)
    feats_b = np.asarray(feats_b, dtype=np.float32)
    W1 = np.asarray(W1, dtype=np.float32)
    b1 = np.asarray(b1, dtype=np.float32)
    W2 = np.asarray(W2, dtype=np.float32)
    b2 = np.asarray(b2, dtype=np.float32)

    if np.any(b1 != 0.0):
        # device pipeline folds dw through relu; exact only for b1 == 0
        return _reference_numpy(
            batch_a, coords_a, feats_a, batch_b, coords_b, feats_b, W1, b1, W2, b2
        )

    in_maps, meta = _host_prep(
        batch_a, coords_a, batch_b, coords_b, feats_b, W1, b1, W2, b2
    )
    if in_maps is None:
        return _reference_numpy(
            batch_a, coords_a, feats_a, batch_b, coords_b, feats_b, W1, b1, W2, b2
        )

    nc = get_program()
    trace = bool(int(os.environ.get("KERNEL_TRACE", "0")))
    if trace:
        _ensure_ntff_hook()
    res = run_bass_kernel_spmd(
        nc, in_maps, core_ids=list(range(N_CORES)), trace=trace
    )
    kernel.last_results = res

    fused = np.zeros((len(batch_a), DF), dtype=np.float32)
    for g in range(N_CORES):
        a_idx, na, nb = meta[g]
        if na == 0:
            continue
        out_g = res.results[g]["fusedT"]  # [DF, NPAD]
        if nb == 0:
            # reference: dw=0 rows -> h=0 -> fused = 3*b2
            fused[a_idx] = 3.0 * b2
        else:
            fused[a_idx] = out_g[:, :na].T
    return np.concatenate([feats_a, fused], axis=1)


# revision 29
# speedup vs baseline: 1.1210x; 1.1210x over previous
"""Batched same-batch KNN (top-3) + fused MLP for Trainium2, 8 NeuronCores.

Strategy
--------
Host side (numpy, exact):
  * Stable-group rows of a and b by batch id. Batch g -> core g (B == 8 ==
    n_cores). Within a batch the original relative order is preserved, so
    the device's first-occurrence tie handling matches jax.lax.top_k.
  * Augment voxel coords so the [Na_g, Nb_g] block of NEGATED squared
    distances is one K=5 matmul:
        d'[i,j] = ua_i . vb_j,  ua = [-|xa|^2, 1, 2*xa],  vb = [1, -|xb|^2, xb]
    All quantities are small integers -> fp32-exact, identical to the
    reference's d (up to sign/scale).
Device side (per core, SPMD):
  * R = relu(feats_bg @ W1 + b1) precomputed once for the whole batch
    (valid because dw >= 0 and relu is positively homogeneous:
     relu(dw*t + b1)*dw == dw^2 * relu(t + b1/dw)... exactly:
     h = relu((f*dw) @ W1 + b1) * dw = dw^2 * relu(f@W1 + b1/dw) only if b1==0;
     in general we use h = relu(dw*(f@W1) + b1)*dw and the harness data has
     b1 == 0 which makes h = dw^2 * relu(f@W1). A nonzero b1 is handled by a
     numpy fallback, see kernel()).
  * Per 128-row a-tile: distance matmul -> PSUM, DVE max (top-8 desc) +
    max_index (ties resolved to successive first occurrences == jax),
    dw = relu(0.5 + d'/16384), indirect-DMA gather of 3 R rows, weighted
    sum with dw^2, PE transpose, @W2 (+3*b2), output fused.T slab.
Outputs are scattered back to original row order on host; the feats_a
passthrough half of the concat is host-side assembly.
"""

import os
import numpy as np

import concourse.bass as bass
import concourse.mybir as mybir
import concourse.tile as tile
from concourse import bacc
from concourse.bass import IndirectOffsetOnAxis
from concourse.bass_utils import run_bass_kernel_spmd
from concourse.masks import make_identity

P = 128
NPAD = 1664  # 13 * 128; covers per-batch row counts for Na=Nb=12288, B=8
NT = NPAD // P
DF = 256
TOPK = 3
FULL_SCALE = 128
RCLIP = 0.5
INV_SCALE2 = 1.0 / (FULL_SCALE * FULL_SCALE)
BIG = 1.0e9
N_CORES = 8
NQ = 3  # SWDGE queues used for indirect gathers

_PROGRAM_CACHE = {}


ROWPACK = bool(int(os.environ.get("KERNEL_ROWPACK", "1")))
BATCHGATHER = bool(int(os.environ.get("KERNEL_BATCHGATHER", "1")))


def _build_program():
    """Build the SPMD Bass program (identical on all 8 cores)."""
    nc = bacc.Bacc("TRN2", target_bir_lowering=False, debug=False)
    f32 = mybir.dt.float32

    uaT = nc.dram_tensor("uaT", [5, NPAD], f32, kind="ExternalInput").ap()
    vbT = nc.dram_tensor("vbT", [5, NPAD], f32, kind="ExternalInput").ap()
    fbT = nc.dram_tensor("fbT", [DF, NPAD], f32, kind="ExternalInput").ap()
    w1b1 = nc.dram_tensor("w1b1", [DF + 1, DF], f32, kind="ExternalInput").ap()
    w2 = nc.dram_tensor("w2", [DF, DF], f32, kind="ExternalInput").ap()
    b2c3 = nc.dram_tensor("b2c3", [P, 2], f32, kind="ExternalInput").ap()
    fusedT = nc.dram_tensor("fusedT", [DF, NPAD], f32, kind="ExternalOutput").ap()

    with tile.TileContext(nc) as tc:
        with (
            tc.tile_pool(name="const", bufs=1) as cpool,
            tc.tile_pool(name="dram", bufs=1, space="DRAM") as dpool_dram,
        ):
            # coord operands replicated at partition offsets 0/32/64/96 so the
            # K=5 distance matmuls can run in 4 concurrent PE row-groups
            uaT4 = cpool.tile([101, NPAD], f32)
            vbT4 = cpool.tile([101, NPAD], f32)
            for gofs in (0, 32, 64, 96):
                nc.sync.dma_start(uaT4[gofs : gofs + 5, :], uaT[:])
                nc.sync.dma_start(vbT4[gofs : gofs + 5, :], vbT[:])
            fbT0 = cpool.tile([P, NPAD], f32)
            nc.sync.dma_start(fbT0[:], fbT[0:P, :])
            fbT1 = cpool.tile([P, NPAD], f32)
            nc.sync.dma_start(fbT1[:], fbT[P : 2 * P, :])
            w1k0 = cpool.tile([P, DF], f32)
            nc.sync.dma_start(w1k0[:], w1b1[0:P, :])
            w1k1 = cpool.tile([P, DF], f32)
            nc.sync.dma_start(w1k1[:], w1b1[P : 2 * P, :])

            w2k0 = cpool.tile([P, DF], f32)
            nc.sync.dma_start(w2k0[:], w2[0:P, :])
            w2k1 = cpool.tile([P, DF], f32)
            nc.sync.dma_start(w2k1[:], w2[P : 2 * P, :])
            b2s = cpool.tile([P, 2], f32)
            nc.sync.dma_start(b2s[:], b2c3[:])
            ident = cpool.tile([P, P], f32)
            make_identity(nc, ident[:])
            zcol = cpool.tile([P, 1], f32)
            nc.vector.memset(zcol[:], 0.0)
            halfcol = cpool.tile([P, 1], f32)
            nc.vector.memset(halfcol[:], RCLIP)

            rtab = dpool_dram.tile([NPAD, DF], f32)

            # ---- Phase R: R = relu(feats_bg @ W1 + b1), row-major in DRAM
            with (
                tc.tile_pool(name="psR", bufs=2, space="PSUM") as psR_pool,
                tc.tile_pool(name="rsb", bufs=3) as r_pool,
            ):
                for t in range(NT):
                    sl = bass.ts(t, P)
                    psR = psR_pool.tile([P, DF], f32)
                    # b1 is asserted zero host-side (numpy fallback otherwise),
                    # so no bias term here
                    nc.tensor.matmul(
                        psR[:], lhsT=fbT0[:, sl], rhs=w1k0[:], start=True, stop=False
                    )
                    nc.tensor.matmul(
                        psR[:], lhsT=fbT1[:, sl], rhs=w1k1[:], start=False, stop=True
                    )
                    rt = r_pool.tile([P, DF], f32)
                    nc.scalar.activation(
                        rt[:], psR[:], mybir.ActivationFunctionType.Relu, bias=zcol[:]
                    )
                    nc.sync.dma_start(rtab[sl, :], rt[:])

            # ---- Phase D: distances, top-3, gather, W2.
            # 1-stage sw pipeline with SKEW tiles of slack between issuing a
            # tile's gathers and consuming them; MM2/output processed in
            # groups of 4 tiles (N=512 moving dim) to amortize LDWEIGHTS.
            GRP = 4
            with (
                tc.tile_pool(name="dps", bufs=1, space="PSUM") as d_pool,
                tc.tile_pool(name="tps", bufs=2, space="PSUM") as t_pool,
                tc.tile_pool(name="fps", bufs=2, space="PSUM") as f_pool,
                tc.tile_pool(name="dsb", bufs=2) as dsb_pool,
                tc.tile_pool(name="small", bufs=8) as s_pool,
                tc.tile_pool(name="gat", bufs=8) as g_pool,
                tc.tile_pool(name="accp", bufs=2) as a_pool,
                tc.tile_pool(name="outp", bufs=2) as o_pool,
            ):
                state = {}

                def topk_and_gather(t):
                    sl = bass.ts(t, P)
                    dps = d_pool.tile([P, NPAD], f32)
                    for c, c0 in enumerate(range(0, NPAD, 512)):
                        c1 = min(c0 + 512, NPAD)
                        gofs = 32 * c if ROWPACK else 0
                        nc.tensor.matmul(
                            dps[:, c0:c1],
                            lhsT=uaT4[gofs : gofs + 5, sl],
                            rhs=vbT4[gofs : gofs + 5, c0:c1],
                            start=True,
                            stop=True,
                            tile_position=(gofs, 0),
                        )
                    dsb = dsb_pool.tile([P, NPAD], f32)
                    nc.scalar.copy(dsb[:], dps[:])
                    vals = s_pool.tile([P, 8], f32, tag="vals")
                    nc.vector.max(out=vals[:], in_=dsb[:])
                    idxs = s_pool.tile([P, 8], mybir.dt.uint32, tag="idxs")
                    nc.vector.max_index(out=idxs[:], in_max=vals[:], in_values=dsb[:])
                    dw = s_pool.tile([P, TOPK], f32, tag="dw")
                    nc.scalar.activation(
                        dw[:],
                        vals[:, 0:TOPK],
                        mybir.ActivationFunctionType.Relu,
                        bias=halfcol[:],
                        scale=INV_SCALE2,
                    )
                    dw2 = s_pool.tile([P, TOPK], f32, tag="dw2")
                    nc.vector.tensor_mul(dw2[:], dw[:], dw[:])
                    gs = []
                    for k in range(TOPK):
                        g = g_pool.tile([P, DF], f32, tag=f"g{k}")
                        nc.gpsimd.indirect_dma_start(
                            out=g[:],
                            out_offset=None,
                            in_=rtab[:],
                            in_offset=IndirectOffsetOnAxis(
                                ap=idxs[:, k : k + 1], axis=0
                            ),
                        )
                        gs.append(g)
                    state[t] = (dw2, gs)

                def mlp_group(g0, g1):
                    # tiles g0..g1-1: weighted-sum, transpose, batched MM2
                    ntile = g1 - g0
                    accT0 = a_pool.tile([P, GRP * P], f32, tag="accT0")
                    accT1 = a_pool.tile([P, GRP * P], f32, tag="accT1")
                    for i, s in enumerate(range(g0, g1)):
                        dw2, gs = state.pop(s)
                        acc = a_pool.tile([P, DF], f32, tag="acc")
                        nc.gpsimd.tensor_scalar_mul(acc[:], gs[0][:], dw2[:, 0:1])
                        for k in range(1, TOPK):
                            nc.vector.scalar_tensor_tensor(
                                out=acc[:],
                                in0=gs[k][:],
                                scalar=dw2[:, k : k + 1],
                                in1=acc[:],
                                op0=mybir.AluOpType.mult,
                                op1=mybir.AluOpType.add,
                            )
                        for m, accTm in enumerate((accT0, accT1)):
                            pt = t_pool.tile([P, P], f32)
                            nc.tensor.transpose(
                                out=pt[:],
                                in_=acc[:, m * P : (m + 1) * P],
                                identity=ident[:],
                            )
                            nc.scalar.copy(accTm[:, bass.ts(i, P)], pt[:])
                    csl = slice(g0 * P, g1 * P)
                    for m in range(2):
                        msl = bass.ts(m, P)
                        pf = f_pool.tile([P, GRP * P], f32)
                        nc.tensor.matmul(
                            pf[:, : ntile * P],
                            lhsT=w2k0[:, msl],
                            rhs=accT0[:, : ntile * P],
                            start=True,
                            stop=False,
                        )
                        nc.tensor.matmul(
                            pf[:, : ntile * P],
                            lhsT=w2k1[:, msl],
                            rhs=accT1[:, : ntile * P],
                            start=False,
                            stop=True,
                        )
                        oT = o_pool.tile([P, GRP * P], f32)
                        nc.scalar.activation(
                            oT[:, : ntile * P],
                            pf[:, : ntile * P],
                            mybir.ActivationFunctionType.Identity,
                            bias=b2s[:, m : m + 1],
                        )
                        nc.sync.dma_start(fusedT[msl, csl], oT[:, : ntile * P])

                SKEW = 2
                done = 0
                for t in range(NT + SKEW):
                    if t < NT:
                        topk_and_gather(t)
                    # flush any complete group whose gathers are >= SKEW old
                    while done < NT and (
                        (min(done + GRP, NT) - 1) + SKEW <= t
                    ):
                        g1 = min(done + GRP, NT)
                        mlp_group(done, g1)
                        done = g1
    nc.compile()
    return nc


def get_program():
    if "nc" not in _PROGRAM_CACHE:
        _PROGRAM_CACHE["nc"] = _build_program()
    return _PROGRAM_CACHE["nc"]


def _host_prep(batch_a, coords_a, batch_b, coords_b, feats_b, W1, b1, W2, b2):
    """Group by batch, build per-core input arrays. Returns (in_maps, meta)."""
    pa = np.argsort(batch_a, kind="stable")
    pb = np.argsort(batch_b, kind="stable")
    ca = np.bincount(batch_a, minlength=N_CORES)
    cb = np.bincount(batch_b, minlength=N_CORES)
    oa = np.concatenate([[0], np.cumsum(ca)])
    ob = np.concatenate([[0], np.cumsum(cb)])

    w1b1 = np.concatenate([W1, b1[None, :]], axis=0).astype(np.float32)
    b2c3 = np.ascontiguousarray((3.0 * b2).astype(np.float32).reshape(2, P).T)

    in_maps = []
    meta = []
    for g in range(N_CORES):
        a_idx = pa[oa[g] : oa[g + 1]]
        b_idx = pb[ob[g] : ob[g + 1]]
        na, nb = len(a_idx), len(b_idx)
        if na > NPAD or nb > NPAD or (0 < nb < TOPK):
            return None, None  # shapes outside the compiled envelope -> fallback
        xa = (coords_a[a_idx] // 16).astype(np.float32)
        xb = (coords_b[b_idx] // 16).astype(np.float32)

        uaT = np.zeros((5, NPAD), dtype=np.float32)
        uaT[1, :] = 1.0
        if na > 0:
            uaT[0, :na] = -np.square(xa).sum(1)
            uaT[2:, :na] = (2.0 * xa).T
            # pad a-cols: copy of column 0 (harmless rows, outputs dropped)
            if na < NPAD:
                uaT[:, na:] = uaT[:, :1]

        vbT = np.zeros((5, NPAD), dtype=np.float32)
        vbT[0, :] = 1.0
        vbT[1, :] = -BIG  # pad cols: huge distance, never selected
        if nb > 0:
            vbT[1, :nb] = -np.square(xb).sum(1)
            vbT[2:, :nb] = xb.T

        fbT = np.zeros((DF, NPAD), dtype=np.float32)
        if nb > 0:
            fbT[:, :nb] = feats_b[b_idx].T

        in_maps.append(
            {
                "uaT": uaT,
                "vbT": vbT,
                "fbT": fbT,
                "w1b1": w1b1,
                "w2": np.ascontiguousarray(W2.astype(np.float32)),
                "b2c3": b2c3,
            }
        )
        meta.append((a_idx, na, nb))
    return in_maps, meta


def _reference_numpy(batch_a, coords_a, feats_a, batch_b, coords_b, feats_b,
                     W1, b1, W2, b2):
    """Exact numpy fallback (mirrors reference.py) for out-of-envelope data."""
    xa = (coords_a // 16).astype(np.float32)
    xb = (coords_b // 16).astype(np.float32)
    d = (
        np.square(xa).sum(1)[:, None]
        + np.square(xb).sum(1)[None, :]
        - 2.0 * (xa @ xb.T)
    )
    d = np.clip(d, 0.0, None) / (FULL_SCALE**2)
    same = batch_a[:, None] == batch_b[None, :]
    d = np.where(same, d, np.inf)
    idx = np.argsort(d, axis=1, kind="stable")[:, :TOPK]
    dv = np.take_along_axis(d, idx, axis=1)
    dwt = RCLIP - np.clip(dv, 0.0, RCLIP)
    b_f = feats_b[idx] * dwt[..., None]
    h = np.maximum(b_f @ W1 + b1, 0.0) * dwt[..., None]
    fused = (h @ W2 + b2).sum(axis=1)
    return np.concatenate([feats_a, fused], axis=1).astype(np.float32)


def _ensure_ntff_hook():
    """Install the axon NTFF profile hook (missing antenv.axon_hooks shim)."""
    import sys
    import types

    if "antenv.axon_hooks" in sys.modules:
        return
    try:
        from trn_agent_boot.trn_boot import _ntff_profile_via_ctypes

        hook = _ntff_profile_via_ctypes("/opt/axon/libaxon_pjrt.so")
    except Exception:
        hook = None
    mod = types.ModuleType("antenv.axon_hooks")
    _state = {"hook": hook}
    mod.get_axon_ntff_profile_hook = lambda: _state["hook"]

    def _set(h):
        _state["hook"] = h

    mod.set_axon_ntff_profile_hook = _set
    sys.modules["antenv.axon_hooks"] = mod


def kernel(batch_a, coords_a, feats_a, batch_b, coords_b, feats_b, W1, b1, W2, b2):
    batch_a = np.asarray(batch_a)
    coords_a = np.asarray(coords_a)
    feats_a = np.asarray(feats_a, dtype=np.float32)
    batch_b = np.asarray(batch_b)
    coords_b = np.asarray(coords_b)
    feats_b = np.asarray(feats_b, dtype=np.float32)
    W1 = np.asarray(W1, dtype=np.float32)
    b1 = np.asarray(b1, dtype=np.float32)
    W2 = np.asarray(W2, dtype=np.float32)
    b2 = np.asarray(b2, dtype=np.float32)

    if np.any(b1 != 0.0):
        # device pipeline folds dw through relu; exact only for b1 == 0
        return _reference_numpy(
            batch_a, coords_a, feats_a, batch_b, coords_b, feats_b, W1, b1, W2, b2
        )

    in_maps, meta = _host_prep(
        batch_a, coords_a, batch_b, coords_b, feats_b, W1, b1, W2, b2
    )
    if in_maps is None:
        return _reference_numpy(
            batch_a, coords_a, feats_a, batch_b, coords_b, feats_b, W1, b1, W2, b2
        )

    nc = get_program()
    trace = bool(int(os.environ.get("KERNEL_TRACE", "0")))
    if trace:
        _ensure_ntff_hook()
    res = run_bass_kernel_spmd(
        nc, in_maps, core_ids=list(range(N_CORES)), trace=trace
    )
    kernel.last_results = res

    fused = np.zeros((len(batch_a), DF), dtype=np.float32)
    for g in range(N_CORES):
        a_idx, na, nb = meta[g]
        if na == 0:
            continue
        out_g = res.results[g]["fusedT"]  # [DF, NPAD]
        if nb == 0:
            # reference: dw=0 rows -> h=0 -> fused = 3*b2
            fused[a_idx] = 3.0 * b2
        else:
            fused[a_idx] = out_g[:, :na].T
    return np.concatenate([feats_a, fused], axis=1)


# revision 31
# speedup vs baseline: 1.3932x; 1.2428x over previous
"""Batched same-batch KNN (top-3) + fused MLP for Trainium2, 8 NeuronCores.

Strategy
--------
Host side (numpy, exact):
  * Stable-group rows of a and b by batch id. Batch g -> core g (B == 8 ==
    n_cores). Within a batch the original relative order is preserved, so
    the device's first-occurrence tie handling matches jax.lax.top_k.
  * Augment voxel coords so the [Na_g, Nb_g] block of NEGATED squared
    distances is one K=5 matmul:
        d'[i,j] = ua_i . vb_j,  ua = [-|xa|^2, 1, 2*xa],  vb = [1, -|xb|^2, xb]
    All quantities are small integers -> fp32-exact, identical to the
    reference's d (up to sign/scale).
Device side (per core, SPMD):
  * R = relu(feats_bg @ W1 + b1) precomputed once for the whole batch
    (valid because dw >= 0 and relu is positively homogeneous:
     relu(dw*t + b1)*dw == dw^2 * relu(t + b1/dw)... exactly:
     h = relu((f*dw) @ W1 + b1) * dw = dw^2 * relu(f@W1 + b1/dw) only if b1==0;
     in general we use h = relu(dw*(f@W1) + b1)*dw and the harness data has
     b1 == 0 which makes h = dw^2 * relu(f@W1). A nonzero b1 is handled by a
     numpy fallback, see kernel()).
  * Per 128-row a-tile: distance matmul -> PSUM, DVE max (top-8 desc) +
    max_index (ties resolved to successive first occurrences == jax),
    dw = relu(0.5 + d'/16384), indirect-DMA gather of 3 R rows, weighted
    sum with dw^2, PE transpose, @W2 (+3*b2), output fused.T slab.
Outputs are scattered back to original row order on host; the feats_a
passthrough half of the concat is host-side assembly.
"""

import os
import numpy as np

import concourse.bass as bass
import concourse.mybir as mybir
import concourse.tile as tile
from concourse import bacc
from concourse.bass import IndirectOffsetOnAxis
from concourse.bass_utils import run_bass_kernel_spmd
from concourse.masks import make_identity

P = 128
NPAD = 1664  # 13 * 128; covers per-batch row counts for Na=Nb=12288, B=8
NT = NPAD // P
DF = 256
TOPK = 3
FULL_SCALE = 128
RCLIP = 0.5
INV_SCALE2 = 1.0 / (FULL_SCALE * FULL_SCALE)
BIG = 1.0e9
N_CORES = 8
NQ = 3  # SWDGE queues used for indirect gathers

_PROGRAM_CACHE = {}


ROWPACK = bool(int(os.environ.get("KERNEL_ROWPACK", "1")))
BATCHGATHER = bool(int(os.environ.get("KERNEL_BATCHGATHER", "1")))


def _build_program():
    """Build the SPMD Bass program (identical on all 8 cores)."""
    nc = bacc.Bacc("TRN2", target_bir_lowering=False, debug=False)
    f32 = mybir.dt.float32

    uaT = nc.dram_tensor("uaT", [5, NPAD], f32, kind="ExternalInput").ap()
    vbT = nc.dram_tensor("vbT", [5, NPAD], f32, kind="ExternalInput").ap()
    fbT = nc.dram_tensor("fbT", [DF, NPAD], f32, kind="ExternalInput").ap()
    w1b1 = nc.dram_tensor("w1b1", [DF + 1, DF], f32, kind="ExternalInput").ap()
    w2 = nc.dram_tensor("w2", [DF, DF], f32, kind="ExternalInput").ap()
    b2c3 = nc.dram_tensor("b2c3", [P, 2], f32, kind="ExternalInput").ap()
    fusedT = nc.dram_tensor("fusedT", [DF, NPAD], f32, kind="ExternalOutput").ap()

    with tile.TileContext(nc) as tc:
        with (
            tc.tile_pool(name="const", bufs=1) as cpool,
            tc.tile_pool(name="dram", bufs=1, space="DRAM") as dpool_dram,
        ):
            # coord operands replicated at partition offsets 0/32/64/96 so the
            # K=5 distance matmuls can run in 4 concurrent PE row-groups
            uaT4 = cpool.tile([101, NPAD], f32)
            vbT4 = cpool.tile([101, NPAD], f32)
            for gofs in (0, 32, 64, 96):
                nc.sync.dma_start(uaT4[gofs : gofs + 5, :], uaT[:])
                nc.sync.dma_start(vbT4[gofs : gofs + 5, :], vbT[:])
            fbT0 = cpool.tile([P, NPAD], f32)
            nc.sync.dma_start(fbT0[:], fbT[0:P, :])
            fbT1 = cpool.tile([P, NPAD], f32)
            nc.sync.dma_start(fbT1[:], fbT[P : 2 * P, :])
            w1k0 = cpool.tile([P, DF], f32)
            nc.sync.dma_start(w1k0[:], w1b1[0:P, :])
            w1k1 = cpool.tile([P, DF], f32)
            nc.sync.dma_start(w1k1[:], w1b1[P : 2 * P, :])

            w2k0 = cpool.tile([P, DF], f32)
            nc.sync.dma_start(w2k0[:], w2[0:P, :])
            w2k1 = cpool.tile([P, DF], f32)
            nc.sync.dma_start(w2k1[:], w2[P : 2 * P, :])
            b2s = cpool.tile([P, 2], f32)
            nc.sync.dma_start(b2s[:], b2c3[:])
            ident = cpool.tile([P, P], f32)
            make_identity(nc, ident[:])
            zcol = cpool.tile([P, 1], f32)
            nc.vector.memset(zcol[:], 0.0)
            halfcol = cpool.tile([P, 1], f32)
            nc.vector.memset(halfcol[:], RCLIP)

            rtab = dpool_dram.tile([NPAD, DF], f32)

            # ---- Phase R: R = relu(feats_bg @ W1 + b1), row-major in DRAM
            with (
                tc.tile_pool(name="psR", bufs=2, space="PSUM") as psR_pool,
                tc.tile_pool(name="rsb", bufs=3) as r_pool,
            ):
                for t in range(NT):
                    sl = bass.ts(t, P)
                    psR = psR_pool.tile([P, DF], f32)
                    # b1 is asserted zero host-side (numpy fallback otherwise),
                    # so no bias term here
                    nc.tensor.matmul(
                        psR[:], lhsT=fbT0[:, sl], rhs=w1k0[:], start=True, stop=False
                    )
                    nc.tensor.matmul(
                        psR[:], lhsT=fbT1[:, sl], rhs=w1k1[:], start=False, stop=True
                    )
                    rt = r_pool.tile([P, DF], f32)
                    nc.scalar.activation(
                        rt[:], psR[:], mybir.ActivationFunctionType.Relu, bias=zcol[:]
                    )
                    nc.sync.dma_start(rtab[sl, :], rt[:])

            # ---- Phase D: distances, top-3, gather, W2.
            # 1-stage sw pipeline with SKEW tiles of slack between issuing a
            # tile's gathers and consuming them; MM2/output processed in
            # groups of 4 tiles (N=512 moving dim) to amortize LDWEIGHTS.
            GRP = 4
            with (
                tc.tile_pool(name="dps", bufs=1, space="PSUM") as d_pool,
                tc.tile_pool(name="tps", bufs=2, space="PSUM") as t_pool,
                tc.tile_pool(name="fps", bufs=2, space="PSUM") as f_pool,
                tc.tile_pool(name="dsb", bufs=3) as dsb_pool,
                tc.tile_pool(name="small", bufs=8) as s_pool,
                tc.tile_pool(name="gat", bufs=8) as g_pool,
                tc.tile_pool(name="accp", bufs=2) as a_pool,
                tc.tile_pool(name="outp", bufs=2) as o_pool,
            ):
                state = {}

                def topk_and_gather(t):
                    sl = bass.ts(t, P)
                    dps = d_pool.tile([P, NPAD], f32)
                    for c, c0 in enumerate(range(0, NPAD, 512)):
                        c1 = min(c0 + 512, NPAD)
                        gofs = 32 * c if ROWPACK else 0
                        nc.tensor.matmul(
                            dps[:, c0:c1],
                            lhsT=uaT4[gofs : gofs + 5, sl],
                            rhs=vbT4[gofs : gofs + 5, c0:c1],
                            start=True,
                            stop=True,
                            tile_position=(gofs, 0),
                        )
                    dsb = dsb_pool.tile([P, NPAD], f32)
                    nc.scalar.copy(dsb[:], dps[:])
                    vals = s_pool.tile([P, 8], f32, tag="vals")
                    nc.vector.max(out=vals[:], in_=dsb[:])
                    idxs = s_pool.tile([P, 8], mybir.dt.uint32, tag="idxs")
                    nc.vector.max_index(out=idxs[:], in_max=vals[:], in_values=dsb[:])
                    dw = s_pool.tile([P, TOPK], f32, tag="dw")
                    nc.scalar.activation(
                        dw[:],
                        vals[:, 0:TOPK],
                        mybir.ActivationFunctionType.Relu,
                        bias=halfcol[:],
                        scale=INV_SCALE2,
                    )
                    dw2 = s_pool.tile([P, TOPK], f32, tag="dw2")
                    nc.scalar.activation(
                        dw2[:], dw[:], mybir.ActivationFunctionType.Square
                    )
                    gs = []
                    for k in range(TOPK):
                        g = g_pool.tile([P, DF], f32, tag=f"g{k}")
                        nc.gpsimd.indirect_dma_start(
                            out=g[:],
                            out_offset=None,
                            in_=rtab[:],
                            in_offset=IndirectOffsetOnAxis(
                                ap=idxs[:, k : k + 1], axis=0
                            ),
                        )
                        gs.append(g)
                    state[t] = (dw2, gs)

                def mlp_group(g0, g1):
                    # tiles g0..g1-1: weighted-sum, transpose, batched MM2
                    ntile = g1 - g0
                    accT0 = a_pool.tile([P, GRP * P], f32, tag="accT0")
                    accT1 = a_pool.tile([P, GRP * P], f32, tag="accT1")
                    for i, s in enumerate(range(g0, g1)):
                        dw2, gs = state.pop(s)
                        acc = a_pool.tile([P, DF], f32, tag="acc")
                        nc.scalar.mul(acc[:], gs[0][:], dw2[:, 0:1])
                        for k in range(1, TOPK):
                            nc.vector.scalar_tensor_tensor(
                                out=acc[:],
                                in0=gs[k][:],
                                scalar=dw2[:, k : k + 1],
                                in1=acc[:],
                                op0=mybir.AluOpType.mult,
                                op1=mybir.AluOpType.add,
                            )
                        for m, accTm in enumerate((accT0, accT1)):
                            pt = t_pool.tile([P, P], f32)
                            nc.tensor.transpose(
                                out=pt[:],
                                in_=acc[:, m * P : (m + 1) * P],
                                identity=ident[:],
                            )
                            nc.scalar.copy(accTm[:, bass.ts(i, P)], pt[:])
                    csl = slice(g0 * P, g1 * P)
                    for m in range(2):
                        msl = bass.ts(m, P)
                        pf = f_pool.tile([P, GRP * P], f32)
                        nc.tensor.matmul(
                            pf[:, : ntile * P],
                            lhsT=w2k0[:, msl],
                            rhs=accT0[:, : ntile * P],
                            start=True,
                            stop=False,
                        )
                        nc.tensor.matmul(
                            pf[:, : ntile * P],
                            lhsT=w2k1[:, msl],
                            rhs=accT1[:, : ntile * P],
                            start=False,
                            stop=True,
                        )
                        oT = o_pool.tile([P, GRP * P], f32)
                        nc.scalar.activation(
                            oT[:, : ntile * P],
                            pf[:, : ntile * P],
                            mybir.ActivationFunctionType.Identity,
                            bias=b2s[:, m : m + 1],
                        )
                        nc.sync.dma_start(fusedT[msl, csl], oT[:, : ntile * P])

                SKEW = 2
                done = 0
                for t0 in range(0, NT + SKEW + 1, 2):
                    for t in (t0, t0 + 1):
                        if t < NT:
                            topk_and_gather(t)
                    t = min(t0 + 1, NT + SKEW)
                    # flush any complete group whose gathers are >= SKEW old
                    while done < NT and ((min(done + GRP, NT) - 1) + SKEW <= t):
                        g1 = min(done + GRP, NT)
                        mlp_group(done, g1)
                        done = g1
    nc.compile()
    return nc


def get_program():
    if "nc" not in _PROGRAM_CACHE:
        _PROGRAM_CACHE["nc"] = _build_program()
    return _PROGRAM_CACHE["nc"]


def _host_prep(batch_a, coords_a, batch_b, coords_b, feats_b, W1, b1, W2, b2):
    """Group by batch, build per-core input arrays. Returns (in_maps, meta)."""
    pa = np.argsort(batch_a, kind="stable")
    pb = np.argsort(batch_b, kind="stable")
    ca = np.bincount(batch_a, minlength=N_CORES)
    cb = np.bincount(batch_b, minlength=N_CORES)
    oa = np.concatenate([[0], np.cumsum(ca)])
    ob = np.concatenate([[0], np.cumsum(cb)])

    w1b1 = np.concatenate([W1, b1[None, :]], axis=0).astype(np.float32)
    b2c3 = np.ascontiguousarray((3.0 * b2).astype(np.float32).reshape(2, P).T)

    in_maps = []
    meta = []
    for g in range(N_CORES):
        a_idx = pa[oa[g] : oa[g + 1]]
        b_idx = pb[ob[g] : ob[g + 1]]
        na, nb = len(a_idx), len(b_idx)
        if na > NPAD or nb > NPAD or (0 < nb < TOPK):
            return None, None  # shapes outside the compiled envelope -> fallback
        xa = (coords_a[a_idx] // 16).astype(np.float32)
        xb = (coords_b[b_idx] // 16).astype(np.float32)

        uaT = np.zeros((5, NPAD), dtype=np.float32)
        uaT[1, :] = 1.0
        if na > 0:
            uaT[0, :na] = -np.square(xa).sum(1)
            uaT[2:, :na] = (2.0 * xa).T
            # pad a-cols: copy of column 0 (harmless rows, outputs dropped)
            if na < NPAD:
                uaT[:, na:] = uaT[:, :1]

        vbT = np.zeros((5, NPAD), dtype=np.float32)
        vbT[0, :] = 1.0
        vbT[1, :] = -BIG  # pad cols: huge distance, never selected
        if nb > 0:
            vbT[1, :nb] = -np.square(xb).sum(1)
            vbT[2:, :nb] = xb.T

        fbT = np.zeros((DF, NPAD), dtype=np.float32)
        if nb > 0:
            fbT[:, :nb] = feats_b[b_idx].T

        in_maps.append(
            {
                "uaT": uaT,
                "vbT": vbT,
                "fbT": fbT,
                "w1b1": w1b1,
                "w2": np.ascontiguousarray(W2.astype(np.float32)),
                "b2c3": b2c3,
            }
        )
        meta.append((a_idx, na, nb))
    return in_maps, meta


def _reference_numpy(batch_a, coords_a, feats_a, batch_b, coords_b, feats_b,
                     W1, b1, W2, b2):
    """Exact numpy fallback (mirrors reference.py) for out-of-envelope data."""
    xa = (coords_a // 16).astype(np.float32)
    xb = (coords_b // 16).astype(np.float32)
    d = (
        np.square(xa).sum(1)[:, None]
        + np.square(xb).sum(1)[None, :]
        - 2.0 * (xa @ xb.T)
    )
    d = np.clip(d, 0.0, None) / (FULL_SCALE**2)
    same = batch_a[:, None] == batch_b[None, :]
    d = np.where(same, d, np.inf)
    idx = np.argsort(d, axis=1, kind="stable")[:, :TOPK]
    dv = np.take_along_axis(d, idx, axis=1)
    dwt = RCLIP - np.clip(dv, 0.0, RCLIP)
    b_f = feats_b[idx] * dwt[..., None]
    h = np.maximum(b_f @ W1 + b1, 0.0) * dwt[..., None]
    fused = (h @ W2 + b2).sum(axis=1)
    return np.concatenate([feats_a, fused], axis=1).astype(np.float32)


def _ensure_ntff_hook():
    """Install the axon NTFF profile hook (missing antenv.axon_hooks shim)."""
    import sys
    import types

    if "antenv.axon_hooks" in sys.modules:
        return
    try:
        from trn_agent_boot.trn_boot import _ntff_profile_via_ctypes

        hook = _ntff_profile_via_ctypes("/opt/axon/libaxon_pjrt.so")
    except Exception:
        hook = None
    mod = types.ModuleType("antenv.axon_hooks")
    _state = {"hook": hook}
    mod.get_axon_ntff_profile_hook = lambda: _state["hook"]

    def _set(h):
        _state["hook"] = h

    mod.set_axon_ntff_profile_hook = _set
    sys.modules["antenv.axon_hooks"] = mod


def kernel(batch_a, coords_a, feats_a, batch_b, coords_b, feats_b, W1, b1, W2, b2):
    batch_a = np.asarray(batch_a)
    coords_a = np.asarray(coords_a)
    feats_a = np.asarray(feats_a, dtype=np.float32)
    batch_b = np.asarray(batch_b)
    coords_b = np.asarray(coords_b)
    feats_b = np.asarray(feats_b, dtype=np.float32)
    W1 = np.asarray(W1, dtype=np.float32)
    b1 = np.asarray(b1, dtype=np.float32)
    W2 = np.asarray(W2, dtype=np.float32)
    b2 = np.asarray(b2, dtype=np.float32)

    if np.any(b1 != 0.0):
        # device pipeline folds dw through relu; exact only for b1 == 0
        return _reference_numpy(
            batch_a, coords_a, feats_a, batch_b, coords_b, feats_b, W1, b1, W2, b2
        )

    in_maps, meta = _host_prep(
        batch_a, coords_a, batch_b, coords_b, feats_b, W1, b1, W2, b2
    )
    if in_maps is None:
        return _reference_numpy(
            batch_a, coords_a, feats_a, batch_b, coords_b, feats_b, W1, b1, W2, b2
        )

    nc = get_program()
    trace = bool(int(os.environ.get("KERNEL_TRACE", "0")))
    if trace:
        _ensure_ntff_hook()
    res = run_bass_kernel_spmd(
        nc, in_maps, core_ids=list(range(N_CORES)), trace=trace
    )
    kernel.last_results = res

    fused = np.zeros((len(batch_a), DF), dtype=np.float32)
    for g in range(N_CORES):
        a_idx, na, nb = meta[g]
        if na == 0:
            continue
        out_g = res.results[g]["fusedT"]  # [DF, NPAD]
        if nb == 0:
            # reference: dw=0 rows -> h=0 -> fused = 3*b2
            fused[a_idx] = 3.0 * b2
        else:
            fused[a_idx] = out_g[:, :na].T
    return np.concatenate([feats_a, fused], axis=1)
